# revision 7
# baseline (speedup 1.0000x reference)
"""BertCorrector kernel for 8 TRN2 NeuronCores.

Computes: segment-mean merge of subword encodings (sorted per-row segment
ids) followed by a dense vocab projection:
    merged[b,w,:] = mean_{s: ids[b,s]==w} enc[b,s,:]   (0 if empty)
    logits = merged @ W + b

Strategy (v5):
  * Globally pack the non-empty (sample, word) pairs (~86.5% of B*WMAX)
    into one contiguous axis and split it evenly across the 8 cores at
    word granularity.  Each core gets ~1/8 of the packed words plus the
    contiguous token range feeding them.  With the observed fill rate
    this is 7 word-tiles of 128 per core instead of 8 -> 12.5% fewer
    stage-B matmul columns and output bytes.
  * Stage A (segment sum) runs as enc^T @ onehot on the TensorEngine.
    Because tokens are sorted by packed word id, each 128-token chunk
    only touches a narrow window of packed-word columns; the matmul
    streams just that window (computed from the actual ids at build
    time, unioned over cores) instead of all word columns.  Onehot
    builds run on Vector (fed by one fused iota+aux DMA split across 8
    queues), merged PSUM->SBUF copies on Scalar.
  * Stage B streams 512-wide W slices against the stationary packed
    mergedT, one 1024-wide vocab chunk at a time with W prefetched two
    chunks ahead (inside the loop, so HBM demand stays spread out).
    PSUM results are cast to bf16 during the PSUM->SBUF copy
    (alternating Vector/Scalar) and stored as bf16 with every store
    split 4-ways across DMA queues (stores are descriptor-latency
    bound at ~80ns per 2KiB partition line).  The host upconverts and
    scatters rows back to the dense [B, WMAX, V] f32 output.
"""

import numpy as np
import ml_dtypes

B, S, H = 32, 512, 768
V = 8192
WMAX = 256
NCORES = 8
P = 128
KO = H // P          # 6 hidden chunks
NV = 1024            # vocab chunk width
NCH = V // NV        # 8 vocab chunks
NWARM = 16


def _plan(segment_ids):
    """Pack non-empty words globally, split across cores, compute windows."""
    ids = np.asarray(segment_ids, np.int64)
    tok_pid = np.empty((B, S), np.int64)    # global packed word id per token
    packed_rows = []                        # global row index b*WMAX+w per packed word
    counts = []
    base = 0
    for b in range(B):
        u, inv_idx, cnt = np.unique(ids[b], return_inverse=True, return_counts=True)
        tok_pid[b] = base + inv_idx
        packed_rows.append(b * WMAX + u)
        counts.append(cnt)
        base += len(u)
    T = base
    packed_rows = np.concatenate(packed_rows)
    counts = np.concatenate(counts).astype(np.float64)
    flat_pid = tok_pid.ravel()              # nondecreasing

    wbound = np.array([round(c * T / NCORES) for c in range(NCORES + 1)])
    tbound = np.searchsorted(flat_pid, wbound)
    assert tbound[0] == 0 and tbound[-1] == B * S

    nwords = wbound[1:] - wbound[:-1]
    ntoks = tbound[1:] - tbound[:-1]
    WP = int(-(-nwords.max() // P) * P)     # padded packed words per core
    KC = int(-(-ntoks.max() // P))          # token chunks per core
    PTW = WP // 2                           # psum tile width (<=512)
    assert PTW <= 512

    # per-chunk packed-word windows, unioned over cores
    wins = []
    for kc in range(KC):
        lo, hi = WP, 0
        for c in range(NCORES):
            a = tbound[c] + kc * P
            bnd = min(tbound[c] + (kc + 1) * P, tbound[c + 1])
            if a >= bnd:
                continue
            loc = flat_pid[a:bnd] - wbound[c]
            lo = min(lo, int(loc.min()))
            hi = max(hi, int(loc.max()) + 1)
        wins.append((lo, hi) if lo < hi else None)

    return dict(
        flat_pid=flat_pid, wbound=wbound, tbound=tbound,
        packed_rows=packed_rows, counts=counts, T=T,
        WP=WP, KC=KC, PTW=PTW, wins=wins,
    )


def _mm_plan(plan):
    """Stage-A matmul schedule: per kc, list of (tile, col_lo, col_hi, start).

    Column ranges are relative to the packed axis [0, WP); tile t covers
    [t*PTW, (t+1)*PTW).  The first matmul touching a psum tile streams the
    full tile width with start=True so every element gets initialized.
    """
    WP, PTW, wins, KC = plan["WP"], plan["PTW"], plan["wins"], plan["KC"]
    first = {0: None, 1: None}
    for kc in range(KC):
        if wins[kc] is None:
            continue
        lo, hi = wins[kc]
        for t in (0, 1):
            if lo < (t + 1) * PTW and hi > t * PTW and first[t] is None:
                first[t] = kc
    sched = []
    for kc in range(KC):
        items = []
        if wins[kc] is not None:
            lo, hi = wins[kc]
            for t in (0, 1):
                tl, th = t * PTW, (t + 1) * PTW
                if lo < th and hi > tl:
                    if first[t] == kc:
                        items.append((t, tl, th, True))
                    else:
                        items.append((t, max(lo, tl), min(hi, th), False))
        sched.append(items)
    last = {0: None, 1: None}
    for kc in range(KC):
        for (t, _, _, _) in sched[kc]:
            last[t] = kc
    return sched, last


def _build_program(plan):
    import concourse.mybir as mybir
    from concourse import bacc
    from concourse.tile import TileContext

    bf16 = mybir.dt.bfloat16
    f32 = mybir.dt.float32

    WP, KC, PTW = plan["WP"], plan["KC"], plan["PTW"]
    NWT = WP // P
    sched, last = _mm_plan(plan)

    nc = bacc.Bacc()
    # fused constants: [:, :WP] iota row, [:, WP:WP+KC] ids, [:, WP+KC:] inv
    aux_d = nc.dram_tensor("aux", [P, WP + 2 * KC], f32, kind="ExternalInput")
    enc_d = nc.dram_tensor("enc", [KC, P, H], bf16, kind="ExternalInput")
    w_d = nc.dram_tensor("wmat", [P, KO, V], bf16, kind="ExternalInput")
    out_d = nc.dram_tensor("out", [WP, V], bf16, kind="ExternalOutput")

    with TileContext(nc) as tc:
        with (
            tc.tile_pool(name="persist", bufs=1) as persist,
            tc.tile_pool(name="wp", bufs=3) as wpool,
            tc.tile_pool(name="outp", bufs=4) as outp,
            tc.tile_pool(name="psA", bufs=5, space="PSUM") as psA,
            tc.tile_pool(name="psB", bufs=3, space="PSUM") as psB,
        ):
            # ---- head DMAs: fused iota+aux split 8-way, then enc, then W
            aux_sb = persist.tile([P, WP + 2 * KC], f32)
            for i in range(8):
                nc.sync.dma_start(
                    out=aux_sb[i * 16:(i + 1) * 16],
                    in_=aux_d[i * 16:(i + 1) * 16],
                )
            enc_sb = persist.tile([P, KC, H], bf16)

            def load_enc(kc, ways):
                step = P // ways
                for i in range(ways):
                    nc.sync.dma_start(
                        out=enc_sb[i * step:(i + 1) * step, kc],
                        in_=enc_d[kc, i * step:(i + 1) * step],
                    )

            load_enc(0, 2)
            load_enc(1, 2)
            for kc in range(2, KC):
                load_enc(kc, 1)

            w_tiles = {}

            def load_w(n, ways=2):
                if n < NCH:
                    t = wpool.tile([P, KO, NV], bf16, tag="w")
                    step = P // ways
                    for ko in range(KO):
                        for i in range(ways):
                            nc.sync.dma_start(
                                out=t[i * step:(i + 1) * step, ko],
                                in_=w_d[i * step:(i + 1) * step, ko,
                                        n * NV:(n + 1) * NV],
                            )
                    w_tiles[n] = t

            load_w(0)
            load_w(1)

            # ---- PE warmup (gated only on the gpsimd memset) ----
            warm_sb = persist.tile([P, P], bf16)
            nc.gpsimd.memset(warm_sb[:], 0.0)
            warm_ps = psB.tile([P, 512], f32, tag="psB")
            for _ in range(NWARM):
                nc.tensor.matmul(
                    warm_ps[:, :64], lhsT=warm_sb[:], rhs=warm_sb[:, :64],
                    start=True, stop=True,
                )

            # ---- onehot tiles (Vector): oneh[tok, col] = (iota==pid)*inv
            oneh = {}
            for kc in range(KC):
                if not sched[kc]:
                    continue
                lo = min(cl for (_, cl, _, _) in sched[kc])
                hi = max(ch for (_, _, ch, _) in sched[kc])
                t = persist.tile([P, hi - lo], bf16, name=f"oneh{kc}")
                nc.vector.tensor_scalar(
                    out=t[:],
                    in0=aux_sb[:, lo:hi],
                    scalar1=aux_sb[:, WP + kc:WP + kc + 1],
                    scalar2=aux_sb[:, WP + KC + kc:WP + KC + kc + 1],
                    op0=mybir.AluOpType.is_equal,
                    op1=mybir.AluOpType.mult,
                )
                oneh[kc] = (t, lo)

            # ---- stage A: mergedT[h, packed_w] = enc^T @ onehot ----
            mergedT = persist.tile([P, KO, WP], bf16)
            for ko in range(KO):
                pts = [psA.tile([P, PTW], f32, tag="psA", name=f"pa{ko}_{t}")
                       for t in (0, 1)]
                for kc in range(KC):
                    if not sched[kc]:
                        continue
                    ot, obase = oneh[kc]
                    for (t, cl, ch, st) in sched[kc]:
                        nc.tensor.matmul(
                            pts[t][:, cl - t * PTW:ch - t * PTW],
                            lhsT=enc_sb[:, kc, ko * P:(ko + 1) * P],
                            rhs=ot[:, cl - obase:ch - obase],
                            start=st,
                            stop=(kc == last[t]),
                        )
                for t in (0, 1):
                    nc.scalar.copy(
                        out=mergedT[:, ko, t * PTW:(t + 1) * PTW], in_=pts[t][:],
                    )

            # ---- stage B: out[w, v] = mergedT^T @ W, bf16 out ----
            for n in range(NCH):
                load_w(n + 2)
                w_sb = w_tiles.pop(n)
                for wt in range(NWT):
                    st = outp.tile([P, NV], bf16, tag="out")
                    for hf in range(NV // 512):
                        pt = psB.tile([P, 512], f32, tag="psB")
                        for ko in range(KO):
                            nc.tensor.matmul(
                                pt[:],
                                lhsT=mergedT[:, ko, wt * P:(wt + 1) * P],
                                rhs=w_sb[:, ko, hf * 512:(hf + 1) * 512],
                                start=(ko == 0),
                                stop=(ko == KO - 1),
                            )
                        if hf % 2 == 0:
                            nc.vector.tensor_copy(
                                out=st[:, hf * 512:(hf + 1) * 512], in_=pt[:])
                        else:
                            nc.scalar.copy(
                                out=st[:, hf * 512:(hf + 1) * 512], in_=pt[:])
                    # stores split 4-way: one store's descriptor chain
                    # (~80ns/row) must stay under the compute cadence and
                    # the final drain short
                    for i in range(4):
                        nc.sync.dma_start(
                            out=out_d[wt * P + i * 32:wt * P + (i + 1) * 32,
                                      n * NV:(n + 1) * NV],
                            in_=st[i * 32:(i + 1) * 32],
                        )

    nc.finalize()
    return nc


def _prep_inputs(bert_encodings, W, plan):
    flat_pid, wbound, tbound = plan["flat_pid"], plan["wbound"], plan["tbound"]
    counts, WP, KC = plan["counts"], plan["WP"], plan["KC"]

    enc_bf = np.asarray(bert_encodings, dtype=np.float32).reshape(B * S, H)
    enc_bf = enc_bf.astype(ml_dtypes.bfloat16)
    w_bf = (np.asarray(W, dtype=np.float32).astype(ml_dtypes.bfloat16)
            .reshape(KO, P, V).transpose(1, 0, 2))
    w_bf = np.ascontiguousarray(w_bf)

    inv = (1.0 / counts).astype(np.float32)

    in_maps = []
    for c in range(NCORES):
        t0, t1 = int(tbound[c]), int(tbound[c + 1])
        ntok = t1 - t0
        enc_c = np.zeros((KC * P, H), dtype=ml_dtypes.bfloat16)
        enc_c[:ntok] = enc_bf[t0:t1]
        enc_c = enc_c.reshape(KC, P, H)

        ids_inv = np.zeros((KC * P, 2), dtype=np.float32)
        ids_inv[:, 0] = -1.0
        ids_inv[:ntok, 0] = (flat_pid[t0:t1] - wbound[c]).astype(np.float32)
        ids_inv[:ntok, 1] = inv[flat_pid[t0:t1]]
        ids_inv = ids_inv.reshape(KC, P, 2)

        aux = np.empty((P, WP + 2 * KC), dtype=np.float32)
        aux[:, :WP] = np.arange(WP, dtype=np.float32)
        aux[:, WP:WP + KC] = ids_inv[:, :, 0].T
        aux[:, WP + KC:] = ids_inv[:, :, 1].T

        in_maps.append({"aux": aux, "enc": enc_c, "wmat": w_bf})
    return in_maps


def kernel(bert_encodings, segment_ids, W, b, num_words, _trace=False):
    from concourse.bass_utils import run_bass_kernel_spmd

    assert int(num_words) == WMAX
    plan = _plan(segment_ids)
    in_maps = _prep_inputs(bert_encodings, W, plan)
    nc = _build_program(plan)

    core_ids = list(range(NCORES))
    res = run_bass_kernel_spmd(nc, in_maps, core_ids, trace=_trace)

    out = np.zeros((B * WMAX, V), dtype=np.float32)
    wbound, packed_rows = plan["wbound"], plan["packed_rows"]
    for c in core_ids:
        nw = int(wbound[c + 1] - wbound[c])
        rows = np.asarray(res.results[c]["out"][:nw]).astype(np.float32)
        out[packed_rows[wbound[c]:wbound[c + 1]]] = rows
    out = out.reshape(B, WMAX, V)

    bias = np.asarray(b, dtype=np.float32)
    if np.any(bias):
        out = out + bias

    if _trace:
        kernel._last_exec_time_ns = res.exec_time_ns
        kernel._last_result = res
    return out


# revision 8
# speedup vs baseline: 1.4730x; 1.4730x over previous
"""BertCorrector kernel for 8 TRN2 NeuronCores.

Computes: segment-mean merge of subword encodings (sorted per-row segment
ids) followed by a dense vocab projection:
    merged[b,w,:] = mean_{s: ids[b,s]==w} enc[b,s,:]   (0 if empty)
    logits = merged @ W + b

Strategy (v6):
  * Globally pack the non-empty (sample, word) pairs (~86.5% of B*WMAX)
    into one contiguous axis and split it evenly across the 8 cores at
    word granularity.  Each core gets ~1/8 of the packed words plus the
    contiguous token range feeding them.  With the observed fill rate
    this is 7 word-tiles of 128 per core instead of 8 -> 12.5% fewer
    stage-B matmul columns and output bytes.
  * Stage A (segment sum) runs as enc^T @ onehot on the TensorEngine.
    Because tokens are sorted by packed word id, each 128-token chunk
    only touches a narrow window of packed-word columns; the matmul
    streams just that window (computed from the actual ids at build
    time, unioned over cores) instead of all word columns.  Onehot
    builds run on Vector (fed by one fused iota+aux DMA split across 8
    queues), merged PSUM->SBUF copies on Scalar.
  * Stage B streams 512-wide W slices against the stationary packed
    mergedT, one 1024-wide vocab chunk at a time with W prefetched two
    chunks ahead inside the loop.  Each dma_start costs ~660ns of
    serial sequencer time on its issuing engine, so loads (aux, enc,
    W) issue from the Sync engine's HWDGE queue while stores issue
    from the Activation engine's queue -- store triggers wait on the
    PSUM->SBUF copies and must not head-of-line-block W prefetch.
    PSUM results are cast to bf16 during the copy (Vector engine) and
    stored as bf16; the final stores are split across queues to keep
    the drain tail short.  The host upconverts and scatters rows back
    to the dense [B, WMAX, V] f32 output.
"""

import numpy as np
import ml_dtypes

B, S, H = 32, 512, 768
V = 8192
WMAX = 256
NCORES = 8
P = 128
KO = H // P          # 6 hidden chunks
NV = 1024            # vocab chunk width
NCH = V // NV        # 8 vocab chunks
NWARM = 16


def _plan(segment_ids):
    """Pack non-empty words globally, split across cores, compute windows."""
    ids = np.asarray(segment_ids, np.int64)
    tok_pid = np.empty((B, S), np.int64)    # global packed word id per token
    packed_rows = []                        # global row index b*WMAX+w per packed word
    counts = []
    base = 0
    for b in range(B):
        u, inv_idx, cnt = np.unique(ids[b], return_inverse=True, return_counts=True)
        tok_pid[b] = base + inv_idx
        packed_rows.append(b * WMAX + u)
        counts.append(cnt)
        base += len(u)
    T = base
    packed_rows = np.concatenate(packed_rows)
    counts = np.concatenate(counts).astype(np.float64)
    flat_pid = tok_pid.ravel()              # nondecreasing

    wbound = np.array([round(c * T / NCORES) for c in range(NCORES + 1)])
    tbound = np.searchsorted(flat_pid, wbound)
    assert tbound[0] == 0 and tbound[-1] == B * S

    nwords = wbound[1:] - wbound[:-1]
    ntoks = tbound[1:] - tbound[:-1]
    WP = int(-(-nwords.max() // P) * P)     # padded packed words per core
    KC = int(-(-ntoks.max() // P))          # token chunks per core
    PTW = WP // 2                           # psum tile width (<=512)
    assert PTW <= 512

    # per-chunk packed-word windows, unioned over cores
    wins = []
    for kc in range(KC):
        lo, hi = WP, 0
        for c in range(NCORES):
            a = tbound[c] + kc * P
            bnd = min(tbound[c] + (kc + 1) * P, tbound[c + 1])
            if a >= bnd:
                continue
            loc = flat_pid[a:bnd] - wbound[c]
            lo = min(lo, int(loc.min()))
            hi = max(hi, int(loc.max()) + 1)
        wins.append((lo, hi) if lo < hi else None)

    return dict(
        flat_pid=flat_pid, wbound=wbound, tbound=tbound,
        packed_rows=packed_rows, counts=counts, T=T,
        WP=WP, KC=KC, PTW=PTW, wins=wins,
    )


def _mm_plan(plan):
    """Stage-A matmul schedule: per kc, list of (tile, col_lo, col_hi, start).

    Column ranges are relative to the packed axis [0, WP); tile t covers
    [t*PTW, (t+1)*PTW).  The first matmul touching a psum tile streams the
    full tile width with start=True so every element gets initialized.
    """
    WP, PTW, wins, KC = plan["WP"], plan["PTW"], plan["wins"], plan["KC"]
    first = {0: None, 1: None}
    for kc in range(KC):
        if wins[kc] is None:
            continue
        lo, hi = wins[kc]
        for t in (0, 1):
            if lo < (t + 1) * PTW and hi > t * PTW and first[t] is None:
                first[t] = kc
    sched = []
    for kc in range(KC):
        items = []
        if wins[kc] is not None:
            lo, hi = wins[kc]
            for t in (0, 1):
                tl, th = t * PTW, (t + 1) * PTW
                if lo < th and hi > tl:
                    if first[t] == kc:
                        items.append((t, tl, th, True))
                    else:
                        items.append((t, max(lo, tl), min(hi, th), False))
        sched.append(items)
    last = {0: None, 1: None}
    for kc in range(KC):
        for (t, _, _, _) in sched[kc]:
            last[t] = kc
    return sched, last


def _build_program(plan):
    import concourse.mybir as mybir
    from concourse import bacc
    from concourse.tile import TileContext

    bf16 = mybir.dt.bfloat16
    f32 = mybir.dt.float32

    WP, KC, PTW = plan["WP"], plan["KC"], plan["PTW"]
    NWT = WP // P
    sched, last = _mm_plan(plan)

    nc = bacc.Bacc()
    # fused constants: [:, :WP] iota row, [:, WP:WP+KC] ids, [:, WP+KC:] inv
    aux_d = nc.dram_tensor("aux", [P, WP + 2 * KC], f32, kind="ExternalInput")
    enc_d = nc.dram_tensor("enc", [KC, P, H], bf16, kind="ExternalInput")
    w_d = nc.dram_tensor("wmat", [P, KO, V], bf16, kind="ExternalInput")
    out_d = nc.dram_tensor("out", [WP, V], bf16, kind="ExternalOutput")

    with TileContext(nc) as tc:
        with (
            tc.tile_pool(name="persist", bufs=1) as persist,
            tc.tile_pool(name="wp", bufs=3) as wpool,
            tc.tile_pool(name="outp", bufs=4) as outp,
            tc.tile_pool(name="psA", bufs=5, space="PSUM") as psA,
            tc.tile_pool(name="psB", bufs=3, space="PSUM") as psB,
        ):
            # ---- head DMAs: fused iota+aux split 8-way, then enc, then W
            aux_sb = persist.tile([P, WP + 2 * KC], f32)
            for i in range(4):
                nc.sync.dma_start(
                    out=aux_sb[i * 32:(i + 1) * 32],
                    in_=aux_d[i * 32:(i + 1) * 32],
                )
            enc_sb = persist.tile([P, KC, H], bf16)

            def load_enc(kc, ways):
                step = P // ways
                for i in range(ways):
                    nc.sync.dma_start(
                        out=enc_sb[i * step:(i + 1) * step, kc],
                        in_=enc_d[kc, i * step:(i + 1) * step],
                    )

            load_enc(0, 2)
            load_enc(1, 2)
            for kc in range(2, KC):
                load_enc(kc, 1)

            w_tiles = {}

            def load_w(n):
                if n < NCH:
                    t = wpool.tile([P, KO, NV], bf16, tag="w")
                    for ko in range(KO):
                        nc.sync.dma_start(
                            out=t[:, ko],
                            in_=w_d[:, ko, n * NV:(n + 1) * NV],
                        )
                    w_tiles[n] = t

            load_w(0)
            load_w(1)

            # ---- PE warmup (gated only on the gpsimd memset) ----
            warm_sb = persist.tile([P, P], bf16)
            nc.gpsimd.memset(warm_sb[:], 0.0)
            warm_ps = psB.tile([P, 512], f32, tag="psB")
            for _ in range(NWARM):
                nc.tensor.matmul(
                    warm_ps[:, :64], lhsT=warm_sb[:], rhs=warm_sb[:, :64],
                    start=True, stop=True,
                )

            # ---- onehot tiles (Vector): oneh[tok, col] = (iota==pid)*inv
            oneh = {}
            for kc in range(KC):
                if not sched[kc]:
                    continue
                lo = min(cl for (_, cl, _, _) in sched[kc])
                hi = max(ch for (_, _, ch, _) in sched[kc])
                t = persist.tile([P, hi - lo], bf16, name=f"oneh{kc}")
                nc.vector.tensor_scalar(
                    out=t[:],
                    in0=aux_sb[:, lo:hi],
                    scalar1=aux_sb[:, WP + kc:WP + kc + 1],
                    scalar2=aux_sb[:, WP + KC + kc:WP + KC + kc + 1],
                    op0=mybir.AluOpType.is_equal,
                    op1=mybir.AluOpType.mult,
                )
                oneh[kc] = (t, lo)

            # ---- stage A: mergedT[h, packed_w] = enc^T @ onehot ----
            mergedT = persist.tile([P, KO, WP], bf16)
            for ko in range(KO):
                pts = [psA.tile([P, PTW], f32, tag="psA", name=f"pa{ko}_{t}")
                       for t in (0, 1)]
                for kc in range(KC):
                    if not sched[kc]:
                        continue
                    ot, obase = oneh[kc]
                    for (t, cl, ch, st) in sched[kc]:
                        nc.tensor.matmul(
                            pts[t][:, cl - t * PTW:ch - t * PTW],
                            lhsT=enc_sb[:, kc, ko * P:(ko + 1) * P],
                            rhs=ot[:, cl - obase:ch - obase],
                            start=st,
                            stop=(kc == last[t]),
                        )
                for t in (0, 1):
                    nc.vector.tensor_copy(
                        out=mergedT[:, ko, t * PTW:(t + 1) * PTW], in_=pts[t][:],
                    )

            # ---- stage B: out[w, v] = mergedT^T @ W, bf16 out ----
            for n in range(NCH):
                load_w(n + 2)
                w_sb = w_tiles.pop(n)
                for wt in range(NWT):
                    st = outp.tile([P, NV], bf16, tag="out")
                    for hf in range(NV // 512):
                        pt = psB.tile([P, 512], f32, tag="psB")
                        for ko in range(KO):
                            nc.tensor.matmul(
                                pt[:],
                                lhsT=mergedT[:, ko, wt * P:(wt + 1) * P],
                                rhs=w_sb[:, ko, hf * 512:(hf + 1) * 512],
                                start=(ko == 0),
                                stop=(ko == KO - 1),
                            )
                        nc.vector.tensor_copy(
                            out=st[:, hf * 512:(hf + 1) * 512], in_=pt[:])
                    if n == NCH - 1 and wt >= NWT - 2:
                        for i in range(4):
                            nc.scalar.dma_start(
                                out=out_d[wt * P + i * 32:wt * P + (i + 1) * 32,
                                          n * NV:(n + 1) * NV],
                                in_=st[i * 32:(i + 1) * 32],
                            )
                    else:
                        nc.scalar.dma_start(
                            out=out_d[wt * P:(wt + 1) * P, n * NV:(n + 1) * NV],
                            in_=st[:],
                        )

    nc.finalize()
    return nc


def _prep_inputs(bert_encodings, W, plan):
    flat_pid, wbound, tbound = plan["flat_pid"], plan["wbound"], plan["tbound"]
    counts, WP, KC = plan["counts"], plan["WP"], plan["KC"]

    enc_bf = np.asarray(bert_encodings, dtype=np.float32).reshape(B * S, H)
    enc_bf = enc_bf.astype(ml_dtypes.bfloat16)
    w_bf = (np.asarray(W, dtype=np.float32).astype(ml_dtypes.bfloat16)
            .reshape(KO, P, V).transpose(1, 0, 2))
    w_bf = np.ascontiguousarray(w_bf)

    inv = (1.0 / counts).astype(np.float32)

    in_maps = []
    for c in range(NCORES):
        t0, t1 = int(tbound[c]), int(tbound[c + 1])
        ntok = t1 - t0
        enc_c = np.zeros((KC * P, H), dtype=ml_dtypes.bfloat16)
        enc_c[:ntok] = enc_bf[t0:t1]
        enc_c = enc_c.reshape(KC, P, H)

        ids_inv = np.zeros((KC * P, 2), dtype=np.float32)
        ids_inv[:, 0] = -1.0
        ids_inv[:ntok, 0] = (flat_pid[t0:t1] - wbound[c]).astype(np.float32)
        ids_inv[:ntok, 1] = inv[flat_pid[t0:t1]]
        ids_inv = ids_inv.reshape(KC, P, 2)

        aux = np.empty((P, WP + 2 * KC), dtype=np.float32)
        aux[:, :WP] = np.arange(WP, dtype=np.float32)
        aux[:, WP:WP + KC] = ids_inv[:, :, 0].T
        aux[:, WP + KC:] = ids_inv[:, :, 1].T

        in_maps.append({"aux": aux, "enc": enc_c, "wmat": w_bf})
    return in_maps


def kernel(bert_encodings, segment_ids, W, b, num_words, _trace=False):
    from concourse.bass_utils import run_bass_kernel_spmd

    assert int(num_words) == WMAX
    plan = _plan(segment_ids)
    in_maps = _prep_inputs(bert_encodings, W, plan)
    nc = _build_program(plan)

    core_ids = list(range(NCORES))
    res = run_bass_kernel_spmd(nc, in_maps, core_ids, trace=_trace)

    out = np.zeros((B * WMAX, V), dtype=np.float32)
    wbound, packed_rows = plan["wbound"], plan["packed_rows"]
    for c in core_ids:
        nw = int(wbound[c + 1] - wbound[c])
        rows = np.asarray(res.results[c]["out"][:nw]).astype(np.float32)
        out[packed_rows[wbound[c]:wbound[c + 1]]] = rows
    out = out.reshape(B, WMAX, V)

    bias = np.asarray(b, dtype=np.float32)
    if np.any(bias):
        out = out + bias

    if _trace:
        kernel._last_exec_time_ns = res.exec_time_ns
        kernel._last_result = res
    return out
